# revision 38
# baseline (speedup 1.0000x reference)
"""Trainium2 Bass kernel for a dense transformer block (B=4, S=2048, E=1024,
H=16 heads, DK=64, FFN 4x) distributed over 8 NeuronCores.

Sharding (fully uniform SPMD, no collectives):
  core c -> batch b = c//2, parity j = c%2.
  The core owns query/FFN tokens at positions j::2 of sequence b (1024
  tokens) and computes K/V over all 2048 tokens of b (duplicated across the
  two cores of a batch; cheaper than a cross-core collective).

Layout: activations are feature-major ([feature, token]) so every matmul
contracts over the partition dim with weights in natural layout.  The host
passes x[b].T and the strided query slice x[b][j::2].T, and transposes the
kernel output back.

Attention: scores are computed transposed ([keys, queries]) per head.
Softmax skips the max-subtraction (scores here have std ~0.1; exp is safe).
Row sums come from a ones-column inside the AV matmul (appended for even
heads, prepended for odd heads so each head's output lands on its own
partition range -- engines cannot move data across partitions).
Causal masking multiplies the exp'd scores with a per-core 0/1 mask tile.
"""

import sys

for _p in ("/opt/trn_rl_repo", "/opt/pypackages"):
    if _p not in sys.path:
        sys.path.append(_p)

import numpy as np
import ml_dtypes

import concourse.bass as bass
import concourse.mybir as mybir
import concourse.tile as tile
from concourse import bacc, bass_utils

F32 = mybir.dt.float32
BF16 = mybir.dt.bfloat16
BF = ml_dtypes.bfloat16
MUL = mybir.AluOpType.mult
ADD = mybir.AluOpType.add
SUB = mybir.AluOpType.subtract
AF = mybir.ActivationFunctionType

P = 128
S = 2048          # full sequence
TOK = 1024        # own tokens per core
E = 1024          # model dim (= D)
EO = E // P       # 8 feature subtiles
H = 16            # heads
DK = 64
FF = 4096
FO = FF // P      # 32
KT = S // P       # 16 key tiles
NG = TOK // 256   # 4 query groups
GQ = 256
CS = S // 512     # 4 chunks of 512 over full seq
CT = TOK // 512   # 2 chunks over own tokens
EPS = 1e-5

_PROG = None


def _emit_ln(nc, tc, pools, x_f32, gb_row, ncols, h_out, tag, cw=512):
    """LayerNorm in feature-major layout.

    x_f32: SBUF [128, EO, ncols] fp32; gb_row: SBUF [2, E] bf16 (g, b);
    h_out: SBUF [128, EO, ncols] bf16.
    Per-token stats via ones-matmuls over the partition dim; per-token
    broadcast factors materialized by rank-1/rank-2 PE matmuls.
    """
    tmp = pools["tmp"]
    onesf, onesb = pools["ones_f"], pools["ones_b"]
    nch = ncols // cw

    with tc.tile_pool(name=f"lnr_{tag}", bufs=1, side="right") as rows:
        r1 = rows.tile([1, ncols], F32, tag="r1")
        r2 = rows.tile([1, ncols], F32, tag="r2")
        r3 = rows.tile([1, ncols], F32, tag="r3")
        rstd_bf = rows.tile([1, ncols], BF16, tag="rstdbf")
        mr = rows.tile([2, ncols], BF16, tag="mr")
        nc.vector.memset(mr[:, :], 1.0)  # row1 stays ones; row0 overwritten

        with tc.tile_pool(name=f"lnsq_{tag}", bufs=2, side="right") as sqp, \
             tc.tile_pool(name=f"lnst_{tag}", bufs=4, space="PSUM") as pst:
            for c in range(nch):
                sl = slice(c * cw, (c + 1) * cw)
                # bf16 copy of x: feeds both the sum matmul (f32 rhs would
                # stream at 1/4 rate) and the square
                xb = sqp.tile([P, EO, cw], BF16, tag="xb")
                nc.vector.tensor_copy(xb[:], x_f32[:, :, sl])
                sq = sqp.tile([P, EO, cw], BF16, tag="sq")
                nc.vector.tensor_tensor(sq[:], xb[:], xb[:], MUL)
                ps_sum = pst.tile([1, cw], F32, tag="st")
                for o in range(EO):
                    nc.tensor.matmul(ps_sum[:], onesb[:, 0:1], xb[:, o, :],
                                     start=(o == 0), stop=(o == EO - 1))
                nc.vector.tensor_copy(r1[:, sl], ps_sum[:])
                ps_sq = pst.tile([1, cw], F32, tag="st")
                for o in range(EO):
                    nc.tensor.matmul(ps_sq[:], onesb[:, 0:1], sq[:, o, :],
                                     start=(o == 0), stop=(o == EO - 1))
                nc.vector.tensor_copy(r2[:, sl], ps_sq[:])

        # r1=sum, r2=sumsq -> r3=mu, r2=rstd, mr[0]=-mu*rstd
        nc.vector.tensor_scalar_mul(r3[:], r1[:], 1.0 / E)
        nc.vector.tensor_tensor(r1[:], r3[:], r1[:], MUL)
        nc.vector.tensor_tensor(r1[:], r2[:], r1[:], SUB)
        nc.vector.tensor_scalar(r1[:], r1[:], 1.0 / E, EPS, MUL, ADD)
        nc.vector.reciprocal(r1[:], r1[:])
        nc.scalar.activation(r2[:], r1[:], AF.Sqrt)
        nc.vector.tensor_copy(rstd_bf[:], r2[:])
        nc.vector.tensor_tensor(r1[:], r3[:], r2[:], MUL)
        nc.vector.tensor_scalar_mul(mr[0:1, :], r1[:], -1.0)

        # h = x * (g[e]*rstd[t]) + (-mu[t]*rstd[t]*g[e] + b[e])
        with tc.tile_pool(name=f"lnbc_{tag}", bufs=4, space="PSUM") as pbc:
            for o in range(EO):
                osl = slice(o * P, (o + 1) * P)
                for c in range(nch):
                    sl = slice(c * cw, (c + 1) * cw)
                    a_ps = pbc.tile([P, cw], F32, tag="bc")
                    c_ps = pbc.tile([P, cw], F32, tag="bc")
                    nc.tensor.matmul(a_ps[:], gb_row[0:1, osl],
                                     rstd_bf[:, sl], start=True, stop=True)
                    nc.tensor.matmul(c_ps[:], gb_row[0:2, osl], mr[:, sl],
                                     start=True, stop=True)
                    t = tmp.tile([P, cw], F32, tag=f"t{cw}")
                    nc.vector.tensor_tensor(t[:], x_f32[:, o, sl], a_ps[:],
                                            MUL)
                    nc.vector.tensor_tensor(h_out[:, o, sl], t[:], c_ps[:],
                                            ADD)


def build_program(skip=()):
    nc = bacc.Bacc("TRN2", target_bir_lowering=False, debug=False)

    xT_d = nc.dram_tensor("xT", [P, EO, S], BF16, kind="ExternalInput")
    xqT_d = nc.dram_tensor("xqT", [P, EO, TOK], F32, kind="ExternalInput")
    wq_d = nc.dram_tensor("wq", [P, EO, E], BF16, kind="ExternalInput")
    wk_d = nc.dram_tensor("wk", [P, EO, E], BF16, kind="ExternalInput")
    wv_d = nc.dram_tensor("wv", [P, EO, E], BF16, kind="ExternalInput")
    wp_d = nc.dram_tensor("wp", [P, EO, E], BF16, kind="ExternalInput")
    w1_d = nc.dram_tensor("w1", [P, EO, FF], BF16, kind="ExternalInput")
    w2_d = nc.dram_tensor("w2", [P, FO, E], BF16, kind="ExternalInput")
    bias_d = nc.dram_tensor("biases", [P, FO + 2 * EO], F32,
                            kind="ExternalInput")  # bp | b1 | b2
    g1_d = nc.dram_tensor("g1b1", [2, E], BF16, kind="ExternalInput")
    g2_d = nc.dram_tensor("g2b2", [2, E], BF16, kind="ExternalInput")
    mask_d = nc.dram_tensor("mask", [P, 4, GQ], BF16, kind="ExternalInput")
    out_d = nc.dram_tensor("outT", [P, EO, TOK], F32, kind="ExternalOutput")

    with tile.TileContext(nc) as tc:
        const = tc.alloc_tile_pool(name="const", bufs=1)
        tmp = tc.alloc_tile_pool(name="tmp", bufs=3)
        pools = {"tmp": tmp}

        ones_f = const.tile([P, 1], F32)
        nc.vector.memset(ones_f[:], 1.0)
        ones_b = const.tile([P, 64], BF16)   # all-ones; rows usable anywhere
        nc.vector.memset(ones_b[:], 1.0)
        pools["ones_f"] = ones_f
        pools["ones_b"] = ones_b

        gb1 = const.tile([2, E], BF16)
        nc.sync.dma_start(gb1[:], g1_d.ap())
        gb2 = const.tile([2, E], BF16)
        nc.sync.dma_start(gb2[:], g2_d.ap())
        bias_sb = const.tile([P, FO + 2 * EO], F32)
        nc.sync.dma_start(bias_sb[:], bias_d.ap())
        bp_pp = bias_sb[:, 0:EO]
        b1_pp = bias_sb[:, EO:EO + FO]
        b2_pp = bias_sb[:, EO + FO:EO + FO + EO]
        mask_sb = const.tile([P, 4, GQ], BF16)
        nc.sync.dma_start(mask_sb[:], mask_d.ap())

        # residual stream for own tokens; allocated at the top of the left
        # stack so its address reuses nothing and the DMA can land early
        xq_pool = tc.alloc_tile_pool(name="xq", bufs=1)
        xq_sb = xq_pool.tile([P, EO, TOK], F32)
        nc.sync.dma_start(xq_sb[:], xqT_d.ap())

        # ---------------- LN1 + QKV, chunk-major -----------------------
        # tokens are host-reordered per core: cols [0:TOK] = own (strided)
        # tokens, cols [TOK:S] = partner-parity tokens.  x arrives BF16 (the
        # residual stream reloads separately in f32 as xqT).  LN1 applies
        # IN PLACE over x, and all of K/Q/V for chunk c are projected as
        # soon as chunk c is normalized, with all weights SBUF-resident.
        kvq_pool = tc.alloc_tile_pool(name="kvq", bufs=1)
        k_sb = kvq_pool.tile([P, EO, S], BF16)
        q_sb = kvq_pool.tile([P, EO, TOK], BF16)
        v_sb = kvq_pool.tile([P, KT, H, DK + 1], BF16)
        # ones column at col DK (used by even heads' fused row-sum)
        nc.vector.memset(v_sb[:, :, :, DK], 1.0)
        if "qkv" in skip:
            nc.vector.memset(k_sb[:], 0.01)
            nc.vector.memset(q_sb[:], 0.01)
            nc.vector.memset(v_sb[:, :, :, 0:DK], 0.01)

        with tc.tile_pool(name="xh", bufs=1, side="right") as xp, \
             tc.tile_pool(name="wv1", bufs=1) as wvp, \
             tc.tile_pool(name="wkq", bufs=4) as wkqp, \
             tc.tile_pool(name="lnr1", bufs=2, side="right") as rows, \
             tc.tile_pool(name="lnsq1", bufs=1, side="right") as sqp, \
             tc.tile_pool(name="lnps1", bufs=2, space="PSUM") as pst, \
             tc.tile_pool(name="lnbc1", bufs=3, space="PSUM") as pbc, \
             tc.tile_pool(name="qkvps", bufs=3, space="PSUM") as pqkv:
            x_sb = xp.tile([P, EO, S], BF16)
            nc.sync.dma_start(x_sb[:], xT_d.ap())
            wv_sb = wvp.tile([P, EO, E], BF16)
            nc.sync.dma_start(wv_sb[:], wv_d.ap())
            for c in (0, 2, 1, 3):   # own chunk 0, then partner chunk 2
                sl = slice(c * 512, (c + 1) * 512)
                if "ln1" not in skip:
                    sq = sqp.tile([P, EO, 512], BF16, tag="sq")
                    nc.vector.tensor_tensor(sq[:], x_sb[:, :, sl],
                                            x_sb[:, :, sl], MUL)
                    r1 = rows.tile([1, 512], F32, tag="r1")
                    r2 = rows.tile([1, 512], F32, tag="r2")
                    r3 = rows.tile([1, 512], F32, tag="r3")
                    rstd_bf = rows.tile([1, 512], BF16, tag="rstdbf")
                    mr = rows.tile([2, 512], BF16, tag="mr")
                    nc.vector.memset(mr[:, :], 1.0)
                    ps_sum = pst.tile([1, 512], F32, tag="st")
                    for o in range(EO):
                        nc.tensor.matmul(ps_sum[:], ones_b[:, 0:1],
                                         x_sb[:, o, sl],
                                         start=(o == 0), stop=(o == EO - 1))
                    nc.vector.tensor_copy(r1[:], ps_sum[:])
                    ps_sq = pst.tile([1, 512], F32, tag="st")
                    for o in range(EO):
                        nc.tensor.matmul(ps_sq[:], ones_b[:, 0:1],
                                         sq[:, o, :],
                                         start=(o == 0), stop=(o == EO - 1))
                    nc.vector.tensor_copy(r2[:], ps_sq[:])
                    # per-chunk row math: mu, rstd, -mu*rstd
                    nc.vector.tensor_scalar_mul(r3[:], r1[:], 1.0 / E)
                    nc.vector.tensor_tensor(r1[:], r3[:], r1[:], MUL)
                    nc.vector.tensor_tensor(r1[:], r2[:], r1[:], SUB)
                    nc.vector.tensor_scalar(r1[:], r1[:], 1.0 / E,
                                            EPS, MUL, ADD)
                    nc.vector.reciprocal(r1[:], r1[:])
                    nc.scalar.activation(r2[:], r1[:], AF.Sqrt)
                    nc.vector.tensor_copy(rstd_bf[:], r2[:])
                    nc.vector.tensor_tensor(r1[:], r3[:], r2[:], MUL)
                    nc.vector.tensor_scalar_mul(mr[0:1, :], r1[:], -1.0)
                    # apply in place: x <- x*(g*rstd) + (-mu*rstd*g + b)
                    for o in range(EO):
                        osl = slice(o * P, (o + 1) * P)
                        a_ps = pbc.tile([P, 512], F32, tag="bc")
                        c_ps = pbc.tile([P, 512], F32, tag="bc")
                        nc.tensor.matmul(a_ps[:], gb1[0:1, osl],
                                         rstd_bf[:], start=True, stop=True)
                        nc.tensor.matmul(c_ps[:], gb1[0:2, osl], mr[:],
                                         start=True, stop=True)
                        t = tmp.tile([P, 512], F32, tag="t512")
                        nc.vector.tensor_tensor(t[:], x_sb[:, o, sl], a_ps[:],
                                                MUL)
                        nc.vector.tensor_tensor(x_sb[:, o, sl], t[:], c_ps[:],
                                                ADD)
                # K/Q/V projections for this normalized chunk
                if "qkv" not in skip:
                    for kk in range(EO):
                        csl = slice(kk * P, (kk + 1) * P)
                        wkt = wkqp.tile([P, EO, P], BF16, tag="wt")
                        nc.sync.dma_start(wkt[:], wk_d.ap()[:, :, csl])
                        ps = pqkv.tile([P, 512], F32, tag="proj")
                        for o in range(EO):
                            nc.tensor.matmul(ps[:], wkt[:, o, :],
                                             x_sb[:, o, sl],
                                             start=(o == 0), stop=(o == EO - 1))
                        nc.scalar.copy(k_sb[:, kk, sl], ps[:])
                    if c in (0, 1):
                        for kk in range(EO):
                            csl = slice(kk * P, (kk + 1) * P)
                            wqt = wkqp.tile([P, EO, P], BF16, tag="wt")
                            nc.sync.dma_start(wqt[:], wq_d.ap()[:, :, csl])
                            ps = pqkv.tile([P, 512], F32, tag="proj")
                            for o in range(EO):
                                nc.tensor.matmul(ps[:], wqt[:, o, :],
                                                 x_sb[:, o, sl],
                                                 start=(o == 0),
                                                 stop=(o == EO - 1))
                            nc.scalar.copy(q_sb[:, kk, sl], ps[:])
                    for kt in range(4 * c, 4 * c + 4):
                        tsl = slice(kt * P, (kt + 1) * P)
                        for dc in range(2):
                            dsl = slice(dc * 512, (dc + 1) * 512)
                            ps = pqkv.tile([P, 512], F32, tag="proj")
                            for o in range(EO):
                                nc.tensor.matmul(ps[:], x_sb[:, o, tsl],
                                                 wv_sb[:, o, dsl],
                                                 start=(o == 0),
                                                 stop=(o == EO - 1))
                            nc.scalar.copy(
                                v_sb[:, kt, dc * 8:(dc + 1) * 8, 0:DK],
                                ps.rearrange("p (h d) -> p h d", d=DK))

        # ---------------- attention -----------------------------------
        o_pool = tc.alloc_tile_pool(name="oc", bufs=1, side="right")
        o_sb = o_pool.tile([P, EO, TOK], BF16)
        with tc.tile_pool(name="exps", bufs=3, side="right") as exp_pool, \
             tc.tile_pool(name="sexp", bufs=1, side="right") as sexp_pool, \
             tc.tile_pool(name="attsm", bufs=2, side="right") as att_sm, \
             tc.tile_pool(name="attps", bufs=2, space="PSUM") as ps_s, \
             tc.tile_pool(name="attpo", bufs=2, space="PSUM") as ps_o, \
             tc.tile_pool(name="attpr", bufs=2, space="PSUM") as ps_r:
            if "attn" in skip:
                nc.vector.memset(o_sb[:], 0.01)
            psum_exp = "psumexp" not in skip
            vflat = v_sb[:].rearrange("p k h d -> p k (h d)")
            # group-major: all heads finish query group g before g+1, so
            # proj can consume o_sb[:, :, qsl(g)] while attention continues
            for g in range(NG if "attn" not in skip else 0):
                # own-parity key tiles 0..2g+1, partner tiles 8..8+2g+1
                kts = list(range(0, 2 * g + 2)) + \
                      list(range(8, 8 + 2 * g + 2))
                nkt = len(kts)
                qsl = slice(g * GQ, (g + 1) * GQ)
                for h in range(H):
                    kk = h >> 1
                    odd = h & 1
                    po = odd * DK
                    osl_o = slice(po, po + DK)
                    # even heads fuse the exp-score row sums into the AV
                    # matmul via the ones column ([v | ones] -> rows 0..64);
                    # odd heads keep a separate M=1 sums matmul at row 32
                    # (base-partition rule forbids output starting at 63/64)
                    vbase = h * (DK + 1)
                    srow = DK if not odd else 32
                    psl = slice(0, DK + 1) if not odd else slice(DK, 2 * DK)
                    es = exp_pool.tile([P, KT, GQ], BF16, tag="exp")
                    ses = None
                    if not psum_exp:
                        ses = sexp_pool.tile([P, KT, GQ], F32, tag="ses")
                    # batch 4 kt-tiles of scores per PSUM tile
                    for kt0 in range(0, nkt if "qk" not in skip else 0, 4):
                        batch = kts[kt0:kt0 + 4]
                        nk = len(batch)
                        sc = ps_s.tile([P, 4, GQ], F32, tag="score")
                        for i, kt in enumerate(batch):
                            ksl = slice(kt * P, (kt + 1) * P)
                            # start=True zeroes the whole 2KB PSUM region
                            # (2 x 256-col f32 slices) -> first writer only
                            nc.tensor.matmul(sc[:, i, :],
                                             k_sb[po:po + DK, kk, ksl],
                                             q_sb[po:po + DK, kk, qsl],
                                             start=(i % 2 == 0),
                                             stop=(i % 2 == 1 or i == nk - 1),
                                             skip_group_check=True)
                        if psum_exp:
                            nc.scalar.activation(es[:, kt0:kt0 + nk, :],
                                                 sc[:, 0:nk, :], AF.Exp)
                        else:
                            nc.vector.tensor_copy(ses[:, kt0:kt0 + nk, :],
                                                  sc[:, 0:nk, :])
                    if "qk" not in skip and not psum_exp:
                        if "exp" not in skip:
                            nc.scalar.activation(es[:, 0:nkt, :],
                                                 ses[:, 0:nkt, :], AF.Exp)
                        else:
                            nc.vector.tensor_copy(es[:, 0:nkt, :],
                                                  ses[:, 0:nkt, :])
                    if "mask" not in skip:
                        # diagonal tiles: own at list pos {2g, 2g+1},
                        # partner diagonal at the last two positions
                        nc.vector.tensor_tensor(es[:, 2 * g:2 * g + 2, :],
                                                es[:, 2 * g:2 * g + 2, :],
                                                mask_sb[:, 0:2], MUL)
                        nc.vector.tensor_tensor(es[:, nkt - 2:nkt, :],
                                                es[:, nkt - 2:nkt, :],
                                                mask_sb[:, 2:4], MUL)
                    oa = ps_o.tile([P, GQ], F32, tag="oacc")
                    if "av" in skip:
                        nc.vector.memset(oa[0:P, :], 1.0)
                        nc.vector.tensor_copy(o_sb[0:P, kk, qsl], oa[0:P, :])
                        continue
                    if not odd:
                        # fused AV: 64 output rows + ones-column row = sums
                        for idx, kt in enumerate(kts):
                            nc.tensor.matmul(oa[psl, :],
                                             vflat[:, kt, vbase:vbase + DK + 1],
                                             es[:, idx, :],
                                             start=(idx == 0),
                                             stop=(idx == nkt - 1))
                    else:
                        for idx, kt in enumerate(kts):
                            nc.tensor.matmul(oa[psl, :],
                                             vflat[:, kt, vbase:vbase + DK],
                                             es[:, idx, :],
                                             start=(idx == 0),
                                             stop=(idx == nkt - 1))
                        for idx in range(nkt):
                            nc.tensor.matmul(oa[32:33, :], ones_b[:, 0:1],
                                             es[:, idx, :],
                                             start=(idx == 0),
                                             stop=(idx == nkt - 1))
                    # ---- normalization (stage PSUM -> SBUF first; walrus
                    # rejects DVE ops with two PSUM operands) ----
                    if "norm" in skip:
                        nc.vector.tensor_copy(o_sb[osl_o, kk, qsl],
                                              oa[osl_o, :])
                        continue
                    ssl = slice(srow, srow + 1)
                    ob = att_sm.tile([P, GQ], F32, tag="ob")
                    if not odd:
                        nc.vector.tensor_copy(ob[psl, :], oa[psl, :])
                    else:
                        nc.vector.tensor_copy(ob[psl, :], oa[psl, :])
                        nc.vector.tensor_copy(ob[ssl, :], oa[ssl, :])
                    rr = att_sm.tile([P, GQ], F32, tag="rr")
                    rrb = att_sm.tile([P, GQ], BF16, tag="rrb")
                    nc.vector.reciprocal(rr[ssl, :], ob[ssl, :])
                    nc.vector.tensor_copy(rrb[ssl, :], rr[ssl, :])
                    rb = ps_r.tile([P, GQ], F32, tag="rb")
                    nc.tensor.matmul(rb[osl_o, :], ones_b[ssl, 0:DK],
                                     rrb[ssl, :], start=True, stop=True)
                    nc.vector.tensor_tensor(o_sb[osl_o, kk, qsl],
                                            ob[osl_o, :], rb[osl_o, :], MUL)
        kvq_pool.release()

        # ---------------- proj + residual + LN2 ------------------------
        x2_pool = tc.alloc_tile_pool(name="x2", bufs=1)
        x2_sb = x2_pool.tile([P, EO, TOK], F32)
        h2_sb = x2_pool.tile([P, EO, TOK], BF16)
        with tc.tile_pool(name="wpt", bufs=8) as wpp, \
             tc.tile_pool(name="prps", bufs=4, space="PSUM") as ppr:
            wpts = []
            for oo in range(EO):
                wpt = wpp.tile([P, EO, P], BF16, tag="wt")
                nc.sync.dma_start(wpt[:], wp_d.ap()[:, :, oo * P:(oo + 1) * P])
                wpts.append(wpt)
            # group-granular so proj overlaps the tail of attention
            for g in range(NG):
                sl = slice(g * GQ, (g + 1) * GQ)
                for oo in range(EO):
                    ps = ppr.tile([P, GQ], F32, tag="proj2")
                    if "proj" in skip:
                        nc.vector.memset(ps[:], 0.0)
                    for s in range(EO if "proj" not in skip else 0):
                        nc.tensor.matmul(ps[:], wpts[oo][:, s, :],
                                         o_sb[:, s, sl],
                                         start=(s == 0), stop=(s == EO - 1))
                    t = tmp.tile([P, GQ], F32, tag="t256")
                    nc.scalar.activation(t[:], ps[:], AF.Identity,
                                         bias=bp_pp[:, oo:oo + 1])
                    nc.vector.tensor_tensor(x2_sb[:, oo, sl], t[:],
                                            xq_sb[:, oo, sl], ADD)
        o_pool.release()
        if "ln2" not in skip:
            _emit_ln(nc, tc, pools, x2_sb, gb2, TOK, h2_sb, "l2")
        else:
            nc.vector.memset(h2_sb[:], 0.01)

        # ---------------- FFN ------------------------------------------
        with tc.tile_pool(name="relu1", bufs=1) as rp, \
             tc.tile_pool(name="w1s", bufs=6) as w1p, \
             tc.tile_pool(name="ffps", bufs=4, space="PSUM") as pff:
            relu1 = rp.tile([P, FO, TOK], BF16)
            if "ffn" in skip:
                nc.vector.memset(relu1[:], 0.01)
            for f in range(FO if "ffn" not in skip else 0):
                fsl = slice(f * P, (f + 1) * P)
                w1t = w1p.tile([P, EO, P], BF16, tag="w1t")
                nc.sync.dma_start(w1t[:], w1_d.ap()[:, :, fsl])
                for c in range(CT):
                    sl = slice(c * 512, (c + 1) * 512)
                    ps = pff.tile([P, 512], F32, tag="ff1")
                    for s in range(EO):
                        nc.tensor.matmul(ps[:], w1t[:, s, :], h2_sb[:, s, sl],
                                         start=(s == 0), stop=(s == EO - 1))
                    nc.scalar.activation(relu1[:, f, sl], ps[:], AF.Relu,
                                         bias=b1_pp[:, f:f + 1])
            with tc.tile_pool(name="w2s", bufs=3) as w2p, \
                 tc.tile_pool(name="outs", bufs=4) as outp:
                for oo in range(EO):
                    osl = slice(oo * P, (oo + 1) * P)
                    nff2 = FO if "ffn2" not in skip else 0
                    w2t = w2p.tile([P, FO, P], BF16, tag="w2t")
                    if nff2:
                        nc.sync.dma_start(w2t[:], w2_d.ap()[:, :, osl])
                    for c in range(CT):
                        sl = slice(c * 512, (c + 1) * 512)
                        ps = pff.tile([P, 512], F32, tag="ff2")
                        if nff2 == 0:
                            nc.vector.memset(ps[:], 0.0)
                        for s in range(nff2):
                            nc.tensor.matmul(ps[:], w2t[:, s, :],
                                             relu1[:, s, sl],
                                             start=(s == 0),
                                             stop=(s == FO - 1))
                        t = tmp.tile([P, 512], F32, tag="t512")
                        nc.scalar.activation(t[:], ps[:], AF.Identity,
                                             bias=b2_pp[:, oo:oo + 1])
                        ot = outp.tile([P, 512], F32, tag="ot")
                        nc.vector.tensor_tensor(ot[:], t[:],
                                                x2_sb[:, oo, sl], ADD)
                        nc.sync.dma_start(out_d.ap()[:, oo, sl], ot[:])
        x2_pool.release()
        xq_pool.release()
        tmp.release()
        const.release()

    nc.compile()
    return nc


def _feat_tile(w, np_dtype):
    """[E_in, N] row-major -> [128, E_in//128, N] (partition, subtile, col)."""
    ei, n = w.shape
    return np.ascontiguousarray(
        w.reshape(ei // P, P, n).transpose(1, 0, 2)).astype(np_dtype)


def _pp(vec):
    """[N] -> [128, N//128] per-partition layout."""
    n = vec.shape[0]
    return np.ascontiguousarray(vec.reshape(n // P, P).T).astype(np.float32)


def _prepare_in_maps(inputs):
    return _make_in_maps(**{k: np.asarray(v) for k, v in inputs.items()})


def _make_in_maps(x, Wq, Wk, Wv, Wp, bp, W1, b1, W2, b2,
                  ln1_g, ln1_b, ln2_g, ln2_b):
    x = np.asarray(x, np.float32)
    scale = 1.0 / np.sqrt(np.float32(E))
    wq_all = np.asarray(Wq, np.float32).transpose(1, 0, 2).reshape(E, H * DK) * scale
    wk_all = np.asarray(Wk, np.float32).transpose(1, 0, 2).reshape(E, H * DK)
    wv_all = np.asarray(Wv, np.float32).transpose(1, 0, 2).reshape(E, H * DK)

    biases = np.concatenate([
        _pp(np.asarray(bp, np.float32)),
        _pp(np.asarray(b1, np.float32)),
        _pp(np.asarray(b2, np.float32))], axis=1)

    shared = {
        "wq": _feat_tile(wq_all, BF),
        "wk": _feat_tile(wk_all, BF),
        "wv": _feat_tile(wv_all, BF),
        "wp": _feat_tile(np.asarray(Wp, np.float32), BF),
        "w1": _feat_tile(np.asarray(W1, np.float32), BF),
        "w2": _feat_tile(np.asarray(W2, np.float32), BF),
        "biases": biases,
        "g1b1": np.stack([np.asarray(ln1_g), np.asarray(ln1_b)]).astype(BF),
        "g2b2": np.stack([np.asarray(ln2_g), np.asarray(ln2_b)]).astype(BF),
    }

    # diagonal-tile masks in the reordered token space: key tiles are
    # [own0, own1, partner0, partner1]; own keys c (pos 2c+j) vs query
    # r (pos 2r+j): c <= r; partner keys c (pos 2c+1-j): c <= r-1+j.
    kap = np.arange(P)[:, None]
    rho = np.arange(GQ)[None, :]
    masks = []
    for j in range(2):
        own0 = (kap <= rho)
        own1 = (kap + P <= rho)
        oth0 = (kap <= rho - 1 + j)
        oth1 = (kap + P <= rho - 1 + j)
        masks.append(np.ascontiguousarray(
            np.stack([own0, own1, oth0, oth1], axis=1)).astype(BF))

    in_maps = []
    for c in range(8):
        b, j = c // 2, c % 2
        # own (strided j::2) tokens first, partner-parity tokens after
        xb_re = np.concatenate([x[b][j::2], x[b][1 - j::2]], axis=0)
        xbT = np.ascontiguousarray(xb_re.T)             # [E, S]
        xqT = np.ascontiguousarray(x[b][j::2].T)        # [E, TOK] f32
        m = dict(shared)
        m["xT"] = np.ascontiguousarray(
            xbT.reshape(EO, P, S).transpose(1, 0, 2)).astype(BF)
        m["xqT"] = np.ascontiguousarray(
            xqT.reshape(EO, P, TOK).transpose(1, 0, 2))
        m["mask"] = masks[j]
        in_maps.append(m)
    return in_maps


def kernel(x, Wq, Wk, Wv, Wp, bp, W1, b1, W2, b2, ln1_g, ln1_b, ln2_g, ln2_b):
    global _PROG
    if _PROG is None:
        _PROG = build_program()
    nc = _PROG

    in_maps = _make_in_maps(x, Wq, Wk, Wv, Wp, bp, W1, b1, W2, b2,
                            ln1_g, ln1_b, ln2_g, ln2_b)
    res = bass_utils.run_bass_kernel_spmd(nc, in_maps, core_ids=list(range(8)))

    out = np.empty((4, S, E), np.float32)
    for c in range(8):
        b, j = c // 2, c % 2
        oT = res.results[c]["outT"]                     # [128, EO, TOK]
        out[b, j::2, :] = oT.transpose(1, 0, 2).reshape(E, TOK).T
    return out

